# revision 17
# baseline (speedup 1.0000x reference)
"""Multi-head causal attention Bass/Tile kernel for TRN2 (v2).

Per-core program (SPMD across 8 cores): each core handles one batch b and
half the heads (HPC=8). Inputs arrive pre-transposed/sliced from the host
in bf16:
  xqT, xkT, xvT : [D, S]   (activations, transposed, bf16)
  wq, wk, wv    : [D, HPC*DK]  (per-core head slice, head-major, bf16)
  bqp, bkp      : [2*DK, HPC//2]  (bias per head-pair column, f32)
  wo            : [HPC*DK, DO]  (slice of Wo rows for these heads, bf16)
Output: out [S, DO] f32 partial (host sums the two head-halves + bias).

v2 design vs v1 (498us -> target ~230us):
- Software-pipelined windows: project QKV for sequence chunk sc+1 and run
  the output projection of block qb-1 *inside* attention block qb's
  instruction stream, so the PE never idles long enough to re-throttle
  (HAM K=4/8 cost ~100us in v1).
- Combined e/o score PSUM tile [128, 2*QB] + one Exp per (pair, kt):
  halves scalar-engine instruction overhead AND makes the two K=64 score
  matmuls depend on the same event, so their tile_position row-packing
  (base partitions 0/64) actually overlaps them in the PE array (2x).
- Causal trimming at 128-col granularity: scores/exp/AV only touch
  columns >= the diagonal, and the mask multiply shrinks to one [128,128]
  triangle per diagonal tile.
- Softmax normalization without DRAM round trips: denominator row ->
  reciprocal_approx_fast (DVE) -> SBUF->SBUF broadcast DMA -> single
  fused multiply PSUM->SBUF (evac + normalize in one pass).
"""

from collections import deque
from contextlib import ExitStack

import numpy as np

import concourse.bass as bass
import concourse.mybir as mybir
import concourse.tile as tile

F32 = mybir.dt.float32
F32R = mybir.dt.float32r
BF16 = mybir.dt.bfloat16
AF = mybir.ActivationFunctionType


def split_multiwaits(nc):
    """This walrus build accepts at most one sync-wait per instruction;
    hoist extra waits onto NOPs placed just before the instruction."""
    n_split = 0
    for fn in nc.m.functions:
        for blk in fn.blocks:
            insts = list(blk.instructions)
            out = []
            for inst in insts:
                si = inst.sync_info
                if si is not None and si.on_wait is not None and len(si.on_wait) > 1:
                    waits = list(si.on_wait)
                    for j, w in enumerate(waits[:-1]):
                        nop = mybir.InstNoOp(name=f"{inst.name}-sw{j}", ins=[], outs=[])
                        nop.engine = inst.engine
                        nop.sync_info = mybir.SyncInfo(on_wait=[w], on_update=[])
                        out.append(nop)
                    inst.sync_info = mybir.SyncInfo(
                        on_wait=[waits[-1]], on_update=list(si.on_update or [])
                    )
                    n_split += 1
                out.append(inst)
            if len(out) != len(insts):
                blk.instructions.clear()
                blk.instructions.extend(out)
    return n_split


def build(S=2048, D=1024, HPC=8, DK=64, DO=1024, QB=512, scale=0.125,
          split=True, mask_engine="vector", fill_per_kt=None):
    n_dt = D // 128          # 8 contraction tiles of D
    n_qb = S // QB           # 4 attention blocks == proj chunks
    n_st = S // 128          # 16 key tiles
    n_pairs = HPC // 2       # 4 head pairs
    HD = HPC * DK            # 512
    n_ht = HD // 128         # 4 ctx tiles (one per pair)
    VA = DK + 1              # V augmented with a ones column
    KPQ = QB // 128          # key tiles per block

    nc = bass.Bass("TRN2", target_bir_lowering=False, debug=False)

    xqT = nc.dram_tensor("xqT", [D, S], BF16, kind="ExternalInput").ap()
    xkT = nc.dram_tensor("xkT", [D, S], BF16, kind="ExternalInput").ap()
    xvT = nc.dram_tensor("xvT", [D, S], BF16, kind="ExternalInput").ap()
    wq = nc.dram_tensor("wq", [D, HD], BF16, kind="ExternalInput").ap()
    wk = nc.dram_tensor("wk", [D, HD], BF16, kind="ExternalInput").ap()
    wv = nc.dram_tensor("wv", [D, HD], BF16, kind="ExternalInput").ap()
    bqp = nc.dram_tensor("bqp", [2 * DK, n_pairs], F32, kind="ExternalInput").ap()
    bkp = nc.dram_tensor("bkp", [2 * DK, n_pairs], F32, kind="ExternalInput").ap()
    wo = nc.dram_tensor("wo", [HD, DO], BF16, kind="ExternalInput").ap()
    tri_in = nc.dram_tensor("tri_in", [128, 128], BF16, kind="ExternalInput").ap()
    vones = nc.dram_tensor("vones", [128, HPC], BF16, kind="ExternalInput").ap()
    out = nc.dram_tensor("out", [S, DO], BF16, kind="ExternalOutput").ap()

    mask_eng = nc.gpsimd if mask_engine == "gpsimd" else nc.vector

    with tile.TileContext(nc) as tc:
        ctx = ExitStack()
        small = ctx.enter_context(tc.tile_pool(name="small", bufs=1))
        wpool = ctx.enter_context(tc.tile_pool(name="w", bufs=1))
        qkpool = ctx.enter_context(tc.tile_pool(name="qk", bufs=1))
        vapool = ctx.enter_context(tc.tile_pool(name="va", bufs=1))
        cspool = ctx.enter_context(tc.tile_pool(name="cs", bufs=1))
        xpool = ctx.enter_context(tc.tile_pool(name="x", bufs=2))
        expool = ctx.enter_context(tc.tile_pool(name="ex", bufs=3))
        rcpool = ctx.enter_context(tc.tile_pool(name="rc", bufs=2))
        bcpool = ctx.enter_context(tc.tile_pool(name="bc", bufs=2))
        evpool = ctx.enter_context(tc.tile_pool(name="ev", bufs=4))
        cxpool = ctx.enter_context(tc.tile_pool(name="cx", bufs=2))
        dscr = ctx.enter_context(tc.tile_pool(name="dscr", bufs=2, space="DRAM"))
        psum = ctx.enter_context(tc.tile_pool(name="ps", bufs=1, space="PSUM"))

        # ---- constants / weights ----
        bq_sb = small.tile([2 * DK, n_pairs], F32, name="bq", tag="bq")
        bk_sb = small.tile([2 * DK, n_pairs], F32, name="bk", tag="bk")
        tri_sb = small.tile([128, 128], BF16, name="tri", tag="tri")
        vones_sb = small.tile([128, HPC], BF16, name="vones", tag="vones")
        nc.sync.dma_start(bq_sb[:], bqp[:])
        nc.sync.dma_start(bk_sb[:], bkp[:])
        nc.sync.dma_start(tri_sb[:], tri_in[:])
        nc.sync.dma_start(vones_sb[:], vones[:])

        # trigger the ACT table load early; using Ln+Exp makes bacc pick the
        # natural_log_exp set once, covering the qb3 scalar reciprocal too.
        dummy = small.tile([1, n_pairs], BF16, name="dummy", tag="dummy")
        nc.scalar.activation(dummy[:], bq_sb[0:1, :], AF.Ln, scale=1.0)
        nc.scalar.activation(dummy[:], bq_sb[0:1, :], AF.Exp, scale=1.0)
        ones64 = small.tile([1, DK], F32, name="ones64", tag="ones64")
        nc.vector.memset(ones64[:], 1.0)

        wq_t = [wpool.tile([128, HD], BF16, name=f"wq{d}", tag=f"wq{d}")
                for d in range(n_dt)]
        wk_t = [wpool.tile([128, HD], BF16, name=f"wk{d}", tag=f"wk{d}")
                for d in range(n_dt)]
        wv_t = [wpool.tile([128, HD], BF16, name=f"wv{d}", tag=f"wv{d}")
                for d in range(n_dt)]
        wo_t = [wpool.tile([128, DO], BF16, name=f"wo{t}", tag=f"wo{t}")
                for t in range(n_ht)]

        def load_w(w_t, w):
            for d, wt in enumerate(w_t):
                nc.sync.dma_start(wt[:], w[d * 128:(d + 1) * 128, :])

        # persistent activation tiles
        qt_t = [[qkpool.tile([2 * DK, QB], BF16, name=f"qt{p}_{sc}", tag=f"qt{p}_{sc}")
                 for sc in range(n_qb)] for p in range(n_pairs)]
        kt_t = [[qkpool.tile([2 * DK, QB], BF16, name=f"kt{p}_{sc}", tag=f"kt{p}_{sc}")
                 for sc in range(n_qb)] for p in range(n_pairs)]
        v_aug = [vapool.tile([128, HPC * VA], BF16, name=f"va{st}", tag=f"va{st}")
                 for st in range(n_st)]
        cst = [[cspool.tile([128, QB], BF16, name=f"cs{t}_{qb}", tag=f"cs{t}_{qb}")
                for qb in range(n_qb)] for t in range(n_ht)]

        xs = {nm: [[None] * n_dt for _ in range(n_qb)] for nm in "qkv"}
        pj_tog = [0]

        def load_x(sc, kinds="qkv"):
            for (xT, nm) in ((xqT, "q"), (xkT, "k"), (xvT, "v")):
                if nm not in kinds:
                    continue
                for d in range(n_dt):
                    xt = xpool.tile([128, QB], BF16, name=f"x{nm}{d}_{sc}",
                                    tag=f"x{nm}{d}")
                    nc.sync.dma_start(xt[:], xT[d * 128:(d + 1) * 128,
                                                sc * QB:(sc + 1) * QB])
                    xs[nm][sc][d] = xt

        def proj_qk_piece(sc, p, w_t, b_sb, dst):
            ps = psum.tile([128, QB], F32, name="pj", tag="pj")
            for d in range(n_dt):
                nc.tensor.matmul(ps[:], w_t[d][:, p * 128:(p + 1) * 128],
                                 xs["q" if dst is qt_t else "k"][sc][d][:],
                                 start=(d == 0), stop=(d == n_dt - 1))
            nc.vector.tensor_scalar_add(dst[p][sc][:], ps[:], b_sb[:, p:p + 1])

        def proj_v_piece(sc, stl):
            ps = psum.tile([128, HD], F32, name="pjv", tag="pj")
            for d in range(n_dt):
                nc.tensor.matmul(ps[:], xs["v"][sc][d][:, stl * 128:(stl + 1) * 128],
                                 wv_t[d][:], start=(d == 0), stop=(d == n_dt - 1))
            st = sc * KPQ + stl
            va3 = v_aug[st][:].rearrange("p (h c) -> p h c", c=VA)
            nc.vector.tensor_copy(va3[:, :, DK:VA],
                                  vones_sb[:].rearrange("p h -> p h ()"))
            nc.vector.tensor_copy(va3[:, :, 0:DK],
                                  ps[:].rearrange("p (h c) -> p h c", c=DK))

        def proj_pieces(sc):
            ops = []
            for p in range(n_pairs):
                ops.append(lambda p=p, sc=sc: proj_qk_piece(sc, p, wq_t, bq_sb, qt_t))
            for p in range(n_pairs):
                ops.append(lambda p=p, sc=sc: proj_qk_piece(sc, p, wk_t, bk_sb, kt_t))
            for stl in range(KPQ):
                ops.append(lambda stl=stl, sc=sc: proj_v_piece(sc, stl))
            return ops

        op_tog = [0]

        def outproj_piece(qb, stl, nck):
            st = qb * KPQ + stl
            ps = psum.tile([128, 512], F32, name="op", tag="op")
            for t in range(n_ht):
                nc.tensor.matmul(ps[:], cst[t][qb][:, stl * 128:(stl + 1) * 128],
                                 wo_t[t][:, nck * 512:(nck + 1) * 512],
                                 start=(t == 0), stop=(t == n_ht - 1))
            ev = evpool.tile([128, 512], BF16, name="ev", tag="ev")
            nc.vector.tensor_copy(ev[:], ps[:])
            nc.sync.dma_start(out[st * 128:(st + 1) * 128,
                                  nck * 512:(nck + 1) * 512], ev[:])

        def outproj_pieces(qb):
            return [lambda qb=qb, stl=stl, nck=nck: outproj_piece(qb, stl, nck)
                    for stl in range(KPQ) for nck in range(DO // 512)]

        def normalize(p, qb, ctx_e, ctx_o):
            # Evacuate ctx PSUM -> SBUF immediately (frees the bank for the
            # next pair's accumulation), then reciprocal + broadcast + the
            # normalize multiply. For the last block the DMA repack chain
            # latency would gate the final output projection, so qb3 uses a
            # scalar-engine reciprocal (exp(-ln d), same ACT table) and a
            # matmul broadcast into the pj PSUM bank (free once proj ends).
            FQ = QB // 128
            for par, ctx_ps, row0 in (("e", ctx_e, 0), ("o", ctx_o, DK)):
                cx = cxpool.tile([VA, QB], F32R, name=f"cx{par}", tag=f"cx{par}")
                nc.vector.tensor_copy(cx[:], ctx_ps[:])
                if qb == n_qb - 1:
                    lnd = rcpool.tile([1, QB], F32, name=f"lnd{par}", tag=f"lnd{par}")
                    nc.scalar.activation(lnd[:], cx[DK:DK + 1, :].bitcast(F32),
                                         AF.Ln, scale=1.0)
                    rcs = rcpool.tile([1, QB], F32R, name=f"rcs{par}", tag=f"rcs{par}")
                    with nc.allow_low_precision(reason="denom recip"):
                        nc.scalar.activation(rcs[:], lnd[:], AF.Exp, scale=-1.0)
                    rbc = psum.tile([DK, QB], F32, name=f"rbc{par}", tag="pj")
                    nc.tensor.matmul(rbc[:], ones64[:].bitcast(F32R), rcs[:],
                                     start=True, stop=True)
                    nc.vector.tensor_mul(cst[p][qb][row0:row0 + DK, :],
                                         cx[0:DK, :], rbc[:])
                    continue
                ds1 = dscr.tile([1, QB], F32, name=f"ds1{par}_{p}_{qb}", tag=f"ds1{par}")
                nc.sync.dma_start(ds1[:], cx[DK:DK + 1, :].bitcast(F32))
                dnp = rcpool.tile([128, FQ], F32, name=f"dnp{par}", tag=f"dnp{par}")
                nc.sync.dma_start(dnp[:], ds1[0, :].rearrange("(p f) -> p f", f=FQ))
                rcp = rcpool.tile([128, FQ], F32R, name=f"rcp{par}", tag=f"rcp{par}")
                with nc.allow_low_precision(reason="denom recip"):
                    nc.vector.reciprocal(rcp[:], dnp[:])
                ds2 = dscr.tile([1, QB], F32R, name=f"ds2{par}_{p}_{qb}", tag=f"ds2{par}")
                nc.sync.dma_start(ds2[0, :].rearrange("(p f) -> p f", f=FQ), rcp[:])
                bc = bcpool.tile([DK, QB], F32R, name=f"bc{par}", tag=f"bc{par}")
                nc.sync.dma_start(bc[:], ds2[:].broadcast_to([DK, QB]))
                nc.vector.tensor_mul(cst[p][qb][row0:row0 + DK, :],
                                     cx[0:DK, :], bc[:])

        # ---------------- prologue ----------------
        # per-kind: issue the weight + x DMAs, then that kind's projections,
        # so the first matmul only waits on the first two small DMAs.
        load_w(wq_t, wq)
        load_x(0, "q")
        for p in range(n_pairs):
            proj_qk_piece(0, p, wq_t, bq_sb, qt_t)
        load_w(wk_t, wk)
        load_x(0, "k")
        for p in range(n_pairs):
            proj_qk_piece(0, p, wk_t, bk_sb, kt_t)
        load_w(wv_t, wv)
        load_x(0, "v")
        for t in range(n_ht):
            nc.sync.dma_start(wo_t[t][:], wo[t * 128:(t + 1) * 128, :])
        load_x(1)

        # ---------------- pipelined windows ----------------
        for qb in range(n_qb):
            fill = deque()
            if qb + 1 < n_qb:
                fill.extend(proj_pieces(qb + 1))
            if qb + 2 < n_qb:
                fill.append(lambda sc=qb + 2: load_x(sc))
            if qb == 2:
                fill.extend(outproj_pieces(0))
            elif qb == 3:
                fill.extend(outproj_pieces(1))
                fill.extend(outproj_pieces(2))
            n_kts = n_pairs * (qb + 1) * KPQ
            n_fill = len(fill)
            drained = 0
            kt_idx = 0
            for p in range(n_pairs):
                he, ho = 2 * p, 2 * p + 1
                ktm = (qb + 1) * KPQ - 1
                ctx_e = psum.tile([VA, QB], F32, name="ctx_e", tag="ctx_e")
                ctx_o = psum.tile([VA, QB], F32, name="ctx_o", tag="ctx_o")
                for kt in range(ktm + 1):
                    if qb == 0 and p == 0 and kt < KPQ:
                        # sc0's V projection, interleaved so the attention
                        # pipeline (scores/exp) spins up under the V matmuls
                        proj_v_piece(0, kt)
                    rel = max(0, kt * 128 - qb * QB)
                    ksc, koff = kt // KPQ, (kt % KPQ) * 128
                    sc_t = psum.tile([128, 2 * QB], F32, name="sc",
                                     tag=f"sc{kt % 2}")
                    nc.tensor.matmul(sc_t[:, rel:QB],
                                     kt_t[p][ksc][0:DK, koff:koff + 128],
                                     qt_t[p][qb][0:DK, rel:QB],
                                     start=True, stop=True)
                    nc.tensor.matmul(sc_t[:, QB + rel:2 * QB],
                                     kt_t[p][ksc][DK:2 * DK, koff:koff + 128],
                                     qt_t[p][qb][DK:2 * DK, rel:QB],
                                     start=True, stop=True)
                    ex_t = expool.tile([128, 2 * QB], BF16, name="ex", tag="ex")
                    if rel == 0:
                        nc.scalar.activation(ex_t[:], sc_t[:], AF.Exp, scale=scale)
                    else:
                        sc3 = sc_t[:].rearrange("p (two f) -> p two f", two=2)
                        ex3 = ex_t[:].rearrange("p (two f) -> p two f", two=2)
                        nc.scalar.activation(ex3[:, :, rel:QB], sc3[:, :, rel:QB],
                                             AF.Exp, scale=scale)
                    if kt * 128 >= qb * QB:
                        mask_eng.tensor_mul(ex_t[:, rel:rel + 128],
                                            ex_t[:, rel:rel + 128], tri_sb[:])
                        mask_eng.tensor_mul(ex_t[:, QB + rel:QB + rel + 128],
                                            ex_t[:, QB + rel:QB + rel + 128],
                                            tri_sb[:])
                    nc.tensor.matmul(ctx_e[:, rel:QB],
                                     v_aug[kt][:, he * VA:(he + 1) * VA],
                                     ex_t[:, rel:QB],
                                     start=(kt == 0), stop=(kt == ktm),
                                     skip_group_check=True)
                    nc.tensor.matmul(ctx_o[:, rel:QB],
                                     v_aug[kt][:, ho * VA:(ho + 1) * VA],
                                     ex_t[:, QB + rel:2 * QB],
                                     start=(kt == 0), stop=(kt == ktm),
                                     skip_group_check=True)
                    kt_idx += 1
                    want = (n_fill * kt_idx + n_kts - 1) // n_kts
                    while drained < want and fill:
                        fill.popleft()()
                        drained += 1
                normalize(p, qb, ctx_e, ctx_o)
            while fill:
                fill.popleft()()
        for op in outproj_pieces(n_qb - 1):
            op()
        ctx.close()

    if split:
        split_multiwaits(nc)
    return nc


def core_inputs(queries, keys, values, Wq, bq, Wk, bk, Wv, bv, Wo, core, n_cores=8,
                HPC=None):
    """Host-side shard prep for one core. core -> (batch, head-group)."""
    import ml_dtypes
    B = queries.shape[0]
    H = Wq.shape[0]
    groups = n_cores // B
    b, hg = core // groups, core % groups
    if HPC is None:
        HPC = H // groups
    h0 = hg * HPC
    DK = Wq.shape[2]
    bf = ml_dtypes.bfloat16

    def wsel(W):
        # [H, D, dk] -> [D, HPC*dk], head-major columns
        return np.ascontiguousarray(
            W[h0:h0 + HPC].transpose(1, 0, 2).reshape(W.shape[1], HPC * DK)
        ).astype(bf)

    def bpairs(bias):
        # [H, dk] -> [2*dk, HPC//2]
        bsel = bias[h0:h0 + HPC].reshape(HPC // 2, 2 * DK)
        return np.ascontiguousarray(bsel.T)

    x_ = np.arange(128)[:, None]
    y_ = np.arange(128)[None, :]
    tri = (y_ - x_ >= 0).astype(bf)
    return {
        "tri_in": tri,
        "vones": np.ones((128, HPC), bf),
        "xqT": np.ascontiguousarray(queries[b].T).astype(bf),
        "xkT": np.ascontiguousarray(keys[b].T).astype(bf),
        "xvT": np.ascontiguousarray(values[b].T).astype(bf),
        "wq": wsel(Wq), "wk": wsel(Wk), "wv": wsel(Wv),
        "bqp": bpairs(bq), "bkp": bpairs(bk),
        "wo": np.ascontiguousarray(Wo[h0 * DK:(h0 + HPC) * DK, :]).astype(bf),
    }


def assemble(results, B, n_cores, bias_total):
    """Sum head-group partials per batch and add the host-side bias."""
    groups = n_cores // B
    outs = []
    for b in range(B):
        acc = results[b * groups]["out"].astype(np.float64)
        for g in range(1, groups):
            acc = acc + results[b * groups + g]["out"]
        outs.append(acc + bias_total)
    return np.stack(outs).astype(np.float32)


# ---------------------------------------------------------------------------
# Harness entry point: full (unsharded) inputs -> full output.
# Shards batch (4) x head-halves (2) across the 8 NeuronCores, runs the Bass
# kernel via run_bass_kernel_spmd, then sums head-half partials per batch on
# the host (+ bias fold: out += bo + bv @ Wo, exact because attention rows
# sum to 1 after normalization).
# ---------------------------------------------------------------------------
_CACHE = {}


def kernel(**inputs):
    from concourse.bass_utils import run_bass_kernel_spmd

    queries = np.asarray(inputs["queries"], np.float32)
    keys = np.asarray(inputs["keys"], np.float32)
    values = np.asarray(inputs["values"], np.float32)
    Wq = np.asarray(inputs["Wq"], np.float32)
    bq = np.asarray(inputs["bq"], np.float32)
    Wk = np.asarray(inputs["Wk"], np.float32)
    bk = np.asarray(inputs["bk"], np.float32)
    Wv = np.asarray(inputs["Wv"], np.float32)
    bv = np.asarray(inputs["bv"], np.float32)
    Wo = np.asarray(inputs["Wo"], np.float32)
    bo = np.asarray(inputs["bo"], np.float32)

    B = queries.shape[0]
    n_cores = 8
    if "nc" not in _CACHE:
        _CACHE["nc"] = build()
    nc = _CACHE["nc"]
    in_maps = [core_inputs(queries, keys, values, Wq, bq, Wk, bk, Wv, bv, Wo,
                           core=c, n_cores=n_cores) for c in range(n_cores)]
    res = run_bass_kernel_spmd(nc, in_maps, list(range(n_cores)))
    bias_total = bo + bv.reshape(-1) @ Wo
    return assemble(res.results, B, n_cores, bias_total)
